# revision 1
# baseline (speedup 1.0000x reference)
"""Trainium2 Bass kernel for ClinicalStateFormationOperator.

Full-input contract: kernel(**inputs) takes the complete (unsharded) numpy
inputs and returns the full [B, T, V, D] output. Internally the work is
sharded across 8 NeuronCores as (batch, head-group): core c handles batch
c//2 and heads (c%2)*4 .. (c%2)*4+3. Each core computes its 4 heads'
attention and the partial output projection; the host sums the two partial
projections per batch and adds the output bias.

Math notes (per core, N = T*V = 1536 tokens, head_dim=64, obs_dim=16).
Scores are computed transposed (keys on partitions, queries free) in ONE
K=128 matmul per [128k x 512q] tile by packing four contraction groups:

  rows  0: 64  kT_h            |  qT_h * scale         (content)
  rows 64: 80  okT_h           |  oqT_h * obs_scale    (observation)
  rows 80:112  [K%32==j]       |  VB_h[Q%32, j]        (variable bias)
  rows112:128  A_hj[s,K]=rtb_h[16j+s-K//32+47] | [(Q//32)%16==s]  (time bias;
               the 16-row basis spans the 16 time bins of q-chunk j, so the
               A rows are re-DMA'd into the k-pack once per (head, q-chunk))

  E^T = exp(scores^T)  (no max-subtraction: |scores| <~ 5, fp32-safe)
  [out^T; denom_rep] = [v | ones_64]^T @ E^T  (ones columns replicate the
        softmax denominator across 64 partitions -> aligned divide)
  OT = out^T * reciprocal(denom_rep)
  y_partial = OT^T_heads @ Wo_rows   (host sums core pairs + bo)

All matmuls run in float32r (full-rate fp32 PE mode, ~1e-4 rel err).
q/k/v/obs biases are zero in this problem's setup_inputs; a with-bias
variant (K=1 bias matmuls into the projection psums) is built only if a
nonzero bias is ever passed.
"""

import numpy as np

import concourse.bass as bass
import concourse.mybir as mybir
import concourse.tile as tile
from concourse.bass_utils import run_bass_kernel_spmd

V = 32
T = 48
D = 512
H = 8
HD = D // H          # 64
OD = 16
B = 4
N = T * V            # 1536
HPC = 4              # heads per core
NCORES = 8
SCALE = 1.0 / np.sqrt(HD)
OBS_SCALE = 1.0 / np.sqrt(OD)

F32 = mybir.dt.float32
F32R = mybir.dt.float32r
EXP = mybir.ActivationFunctionType.Exp

KC = N // 128        # 12 key chunks of 128
QC = N // 512        # 3 query chunks of 512


def _split_waits(nc, max_waits=1):
    """Walrus in this container allows only one sync-wait slot per
    instruction; spill extra waits onto preceding same-engine NoOps."""
    def fix_bb(bb):
        changed = False
        new = []
        for inst in bb.instructions:
            si = inst.sync_info
            if si is not None and len(si.on_wait) > max_waits:
                waits = list(si.on_wait)
                for w in waits[:-max_waits]:
                    new.append(mybir.InstNoOp(
                        name=nc.get_next_instruction_name(),
                        engine=inst.engine, ins=[], outs=[],
                        sync_info=mybir.SyncInfo(on_wait=[w], on_update=[])))
                    changed = True
                si.on_wait = waits[-max_waits:]
            new.append(inst)
        if changed:
            bb.instructions = new
        for sub in getattr(bb, 'blocks', []) or []:
            fix_bb(sub)
    for f in nc.m.functions:
        for bb in f.blocks:
            fix_bb(bb)


def _build(with_bias=False):
    nc = bass.Bass()

    # ---- per-core DRAM I/O (data differs per core, program is SPMD) ----
    fhT = nc.dram_tensor('fhT', [D, N], F32R, kind='ExternalInput')
    foT = nc.dram_tensor('foT', [2, N], F32R, kind='ExternalInput')
    wq = nc.dram_tensor('wq', [D, HPC * HD], F32R, kind='ExternalInput')
    wk = nc.dram_tensor('wk', [D, HPC * HD], F32R, kind='ExternalInput')
    wv = nc.dram_tensor('wv', [D, HPC * HD], F32R, kind='ExternalInput')
    woq = nc.dram_tensor('woq', [2, 128], F32R, kind='ExternalInput')
    wok = nc.dram_tensor('wok', [2, 128], F32R, kind='ExternalInput')
    wo = nc.dram_tensor('wo', [2, 128, D], F32R, kind='ExternalInput')
    # score-bias expansion tables (host-gathered from variable_bias / rtb)
    kstat = nc.dram_tensor('kstat', [32, N], F32R, kind='ExternalInput')
    qstat = nc.dram_tensor('qstat', [HPC, 48, N], F32R, kind='ExternalInput')
    apack = nc.dram_tensor('apack', [HPC, QC, 16, N], F32R,
                           kind='ExternalInput')
    vones = nc.dram_tensor('vones', [128, 64], F32R, kind='ExternalInput')
    if with_bias:
        bqr = nc.dram_tensor('bqr', [1, HPC * HD], F32R, kind='ExternalInput')
        bkr = nc.dram_tensor('bkr', [1, HPC * HD], F32R, kind='ExternalInput')
        bvr = nc.dram_tensor('bvr', [1, HPC * HD], F32R, kind='ExternalInput')
        boqr = nc.dram_tensor('boqr', [1, 128], F32R, kind='ExternalInput')
        bokr = nc.dram_tensor('bokr', [1, 128], F32R, kind='ExternalInput')
        onesd = nc.dram_tensor('onesd', [1, 512], F32R, kind='ExternalInput')
    out = nc.dram_tensor('out', [N, D], F32, kind='ExternalOutput')

    with tile.TileContext(nc) as tc:
        with tc.tile_pool(name='cst', bufs=1) as cst, \
             tc.tile_pool(name='big', bufs=1) as big, \
             tc.tile_pool(name='work', bufs=3) as work, \
             tc.tile_pool(name='et', bufs=3) as etp, \
             tc.tile_pool(name='ps3', bufs=2, space='PSUM') as ps3, \
             tc.tile_pool(name='ps', bufs=2, space='PSUM') as ps:

            # ---- DMA order: wq + fhT chunks first so PE starts early ----
            t_wq = cst.tile([128, 4, HPC * HD], F32R)
            nc.sync.dma_start(t_wq[:], wq[:].rearrange('(o p) n -> p o n', p=128))
            t_fhT = big.tile([128, 4, N], F32R)
            fhT_r = fhT[:].rearrange('(o p) n -> p o n', p=128)
            for kk in range(4):
                nc.sync.dma_start(t_fhT[:, kk, :], fhT_r[:, kk, :])
            t_wk = cst.tile([128, 4, HPC * HD], F32R)
            nc.sync.dma_start(t_wk[:], wk[:].rearrange('(o p) n -> p o n', p=128))
            t_wv = cst.tile([128, 4, HPC * HD], F32R)
            nc.sync.dma_start(t_wv[:], wv[:].rearrange('(o p) n -> p o n', p=128))
            t_foT = cst.tile([2, N], F32R)
            nc.sync.dma_start(t_foT[:], foT[:])
            t_woq = cst.tile([2, 128], F32R)
            nc.sync.dma_start(t_woq[:], woq[:])
            t_wok = cst.tile([2, 128], F32R)
            nc.sync.dma_start(t_wok[:], wok[:])
            t_wo = cst.tile([128, 2, D], F32R)
            nc.sync.dma_start(t_wo[:], wo[:].rearrange('o p n -> p o n'))
            if with_bias:
                t_bq = cst.tile([1, HPC * HD], F32R)
                nc.sync.dma_start(t_bq[:], bqr[:])
                t_bk = cst.tile([1, HPC * HD], F32R)
                nc.sync.dma_start(t_bk[:], bkr[:])
                t_bv = cst.tile([1, HPC * HD], F32R)
                nc.sync.dma_start(t_bv[:], bvr[:])
                t_boq = cst.tile([1, 128], F32R)
                nc.sync.dma_start(t_boq[:], boqr[:])
                t_bok = cst.tile([1, 128], F32R)
                nc.sync.dma_start(t_bok[:], bokr[:])
                t_ones = cst.tile([1, 512], F32R)
                nc.sync.dma_start(t_ones[:], onesd[:])

            # score packs per head [128, N]; static rows DMA'd from tables
            t_qp = [big.tile([128, N], F32R, tag=f'qp{hh}', name=f'qp{hh}')
                    for hh in range(HPC)]
            t_kp = [big.tile([128, N], F32R, tag=f'kp{hh}', name=f'kp{hh}')
                    for hh in range(HPC)]
            for hh in range(HPC):
                nc.sync.dma_start(t_qp[hh][80:128, :], qstat[hh])
                nc.sync.dma_start(t_kp[hh][80:112, :], kstat[:])
            # v in natural layout per head + 64 ones columns for denominators
            t_v = [big.tile([128, KC, 128], F32R, tag=f'v{hh}', name=f'v{hh}')
                   for hh in range(HPC)]
            for hh in range(HPC):
                nc.sync.dma_start(
                    t_v[hh][:, :, 64:128],
                    vones[:, None, :].to_broadcast((128, KC, 64)))
            # attention-out^T pairs (heads 2p, 2p+1 stacked on partitions)
            t_ot = [big.tile([128, N], F32R, tag=f'ot{p}', name=f'ot{p}')
                    for p in range(2)]

            # ---- stage 1: projections ----
            # qT / kT: psum rows = 128 output channels (2 heads), cols = tokens
            # m=0 (heads 0,1) first so stage 2 can begin before m=1 finishes
            def emit_qk(m):
                for (w_t, b_name, pack, sc) in ((t_wq, 'bq', t_qp, SCALE),
                                                (t_wk, 'bk', t_kp, 1.0)):
                    for j in range(QC):
                        p_qt = ps.tile([128, 512], F32, tag='mm', name='p_qt')
                        for kk in range(4):
                            nc.tensor.matmul(
                                p_qt[:], w_t[:, kk, m * 128:(m + 1) * 128],
                                t_fhT[:, kk, j * 512:(j + 1) * 512],
                                start=(kk == 0),
                                stop=(not with_bias and kk == 3))
                        if with_bias:
                            bt = t_bq if b_name == 'bq' else t_bk
                            nc.tensor.matmul(
                                p_qt[:], bt[:, m * 128:(m + 1) * 128],
                                t_ones[:], start=False, stop=True)
                        for s in range(2):
                            hh = 2 * m + s
                            if sc == 1.0:
                                nc.scalar.copy(
                                    pack[hh][0:64, j * 512:(j + 1) * 512],
                                    p_qt[s * 64:(s + 1) * 64, :])
                            else:
                                nc.vector.tensor_scalar_mul(
                                    pack[hh][0:64, j * 512:(j + 1) * 512],
                                    p_qt[s * 64:(s + 1) * 64, :], sc)

            emit_qk(0)
            # oqT / okT: heads padded to 32-row psum boundaries
            for (w_t, b_name, pack, sc) in ((t_woq, 'boq', t_qp, OBS_SCALE),
                                            (t_wok, 'bok', t_kp, 1.0)):
                for j in range(QC):
                    p_o = ps.tile([128, 512], F32, tag='mm', name='p_o')
                    nc.tensor.matmul(p_o[:], w_t[:],
                                     t_foT[:, j * 512:(j + 1) * 512],
                                     start=True, stop=(not with_bias))
                    if with_bias:
                        bt = t_boq if b_name == 'boq' else t_bok
                        nc.tensor.matmul(p_o[:], bt[:], t_ones[:],
                                         start=False, stop=True)
                    for hh in range(HPC):
                        if sc == 1.0:
                            nc.scalar.copy(
                                pack[hh][64:80, j * 512:(j + 1) * 512],
                                p_o[hh * 32:hh * 32 + OD, :])
                        else:
                            nc.vector.tensor_scalar_mul(
                                pack[hh][64:80, j * 512:(j + 1) * 512],
                                p_o[hh * 32:hh * 32 + OD, :], sc)
            # v natural layout: psum [128 tokens, 256 channels] per token chunk
            for kc in range(KC):
                p_v = ps.tile([128, HPC * HD], F32, tag='mm', name='p_v')
                for kk in range(4):
                    nc.tensor.matmul(p_v[:], t_fhT[:, kk, kc * 128:(kc + 1) * 128],
                                     t_wv[:, kk, :], start=(kk == 0),
                                     stop=(not with_bias and kk == 3))
                if with_bias:
                    nc.tensor.matmul(p_v[:], t_ones[:, 0:128], t_bv[:],
                                     start=False, stop=True)
                for hh in range(HPC):
                    nc.vector.tensor_copy(t_v[hh][:, kc, 0:64],
                                          p_v[:, hh * 64:(hh + 1) * 64])
            emit_qk(1)

            # ---- stages 2+3+4, interleaved per q-chunk ----
            for j in range(QC):
                for hh in range(HPC):
                    # time-bias basis rows for this (head, q-chunk)
                    nc.sync.dma_start(t_kp[hh][112:128, :], apack[hh, j])
                    p_ot = ps.tile([128, 512], F32, tag='mm', name='p_ot')
                    for g in range(KC // 3):
                        p_s3 = ps3.tile([128, 3, 512], F32, tag='s3',
                                        name='p_s3')
                        for i3 in range(3):
                            kc = 3 * g + i3
                            nc.tensor.matmul(
                                p_s3[:, i3, :],
                                t_kp[hh][:, kc * 128:(kc + 1) * 128],
                                t_qp[hh][:, j * 512:(j + 1) * 512],
                                start=True, stop=True)
                        t_et = etp.tile([128, 3, 512], F32R, tag='et',
                                        name='t_et')
                        nc.scalar.activation(t_et[:], p_s3[:], EXP)
                        for i3 in range(3):
                            kc = 3 * g + i3
                            nc.tensor.matmul(p_ot[:], t_v[hh][:, kc, :],
                                             t_et[:, i3, :],
                                             start=(kc == 0),
                                             stop=(kc == KC - 1))
                    t_rec = work.tile([64, 512], F32, tag='rec', name='t_rec')
                    nc.vector.reciprocal(t_rec[:], p_ot[64:128, :])
                    nc.vector.tensor_mul(
                        t_ot[hh // 2][(hh % 2) * 64:(hh % 2) * 64 + 64,
                                      j * 512:(j + 1) * 512],
                        p_ot[0:64, :], t_rec[:])
                # partial out-projection for this q-chunk's 4 row blocks
                for qq in range(4):
                    qc = 4 * j + qq
                    p_y = ps.tile([128, D], F32, tag='mm', name='p_y')
                    for p in range(2):
                        nc.tensor.matmul(p_y[:],
                                         t_ot[p][:, qc * 128:(qc + 1) * 128],
                                         t_wo[:, p, :], start=(p == 0),
                                         stop=(p == 1))
                    t_y = work.tile([128, D], F32, tag='y', name='t_y')
                    nc.vector.tensor_copy(t_y[:], p_y[:])
                    nc.sync.dma_start(out[qc * 128:(qc + 1) * 128, :], t_y[:])

    _split_waits(nc)
    return nc


_NC_CACHE = {}


def _get_nc(with_bias=False):
    if with_bias not in _NC_CACHE:
        _NC_CACHE[with_bias] = _build(with_bias)
    return _NC_CACHE[with_bias]


def _pad_obs(a):
    # lay each head's 16 obs channels at a 32-column boundary (PSUM reads
    # must start at 32-partition-aligned offsets)
    out = np.zeros((a.shape[0], 128), np.float32)
    for hh in range(HPC):
        out[:, hh * 32:hh * 32 + OD] = a[:, hh * OD:(hh + 1) * OD]
    return out


def _host_prep(h, observation_state, Wq, bq, Wk, bk, Wv, bv, Wo, bo,
               Woq, boq, Wok, bok, variable_bias, relative_time_bias,
               with_bias=False):
    f32 = np.float32
    h = np.asarray(h, f32)
    obs = np.asarray(observation_state, f32)
    Kidx = np.arange(N)
    tK = Kidx // V                                     # time bin of each token
    kstat = (Kidx[None, :] % V == np.arange(V)[:, None]).astype(f32)
    bq16 = ((Kidx[None, :] // V) % 16 == np.arange(16)[:, None]).astype(f32)

    in_maps = []
    for c in range(NCORES):
        b, hg = divmod(c, 2)
        h0 = hg * HPC
        cs, ce = h0 * HD, (h0 + HPC) * HD
        os_, oe = h0 * OD, (h0 + HPC) * OD
        qstat = np.empty((HPC, 48, N), f32)
        ap = np.empty((HPC, QC, 16, N), f32)
        for hh in range(HPC):
            head = h0 + hh
            vb = np.asarray(variable_bias[head], f32)
            rtb = np.asarray(relative_time_bias[head], f32)
            qstat[hh, :V] = vb[Kidx % V, :].T          # VB_h[Q%32, j]
            qstat[hh, V:] = bq16
            for j in range(QC):
                # A_hj[s, K] = rtb[16j + s - K//32 + 47]
                idx = 16 * j + np.arange(16)[:, None] - tK[None, :] + (T - 1)
                ap[hh, j] = rtb[idx]
        m = {
            'fhT': np.ascontiguousarray(h[b].reshape(N, D).T),
            'foT': np.ascontiguousarray(obs[b].reshape(N, 2).T),
            'wq': np.ascontiguousarray(np.asarray(Wq, f32)[:, cs:ce]),
            'wk': np.ascontiguousarray(np.asarray(Wk, f32)[:, cs:ce]),
            'wv': np.ascontiguousarray(np.asarray(Wv, f32)[:, cs:ce]),
            'woq': _pad_obs(np.asarray(Woq, f32)[:, os_:oe]),
            'wok': _pad_obs(np.asarray(Wok, f32)[:, os_:oe]),
            'wo': np.ascontiguousarray(
                np.asarray(Wo, f32)[cs:ce, :].reshape(2, 128, D)),
            'kstat': kstat,
            'qstat': qstat,
            'apack': ap,
            'vones': np.ones((128, 64), f32),
        }
        if with_bias:
            m.update({
                'bqr': np.ascontiguousarray(np.asarray(bq, f32)[None, cs:ce]),
                'bkr': np.ascontiguousarray(np.asarray(bk, f32)[None, cs:ce]),
                'bvr': np.ascontiguousarray(np.asarray(bv, f32)[None, cs:ce]),
                'boqr': _pad_obs(np.asarray(boq, f32)[None, os_:oe]),
                'bokr': _pad_obs(np.asarray(bok, f32)[None, os_:oe]),
                'onesd': np.ones((1, 512), f32),
            })
        in_maps.append(m)
    return in_maps


def kernel(**inputs):
    with_bias = any(
        np.any(np.asarray(inputs[k])) for k in ('bq', 'bk', 'bv', 'boq', 'bok'))
    nc = _get_nc(with_bias)
    in_maps = _host_prep(**inputs, with_bias=with_bias)
    res = run_bass_kernel_spmd(nc, in_maps, core_ids=list(range(NCORES)))
    bo = np.asarray(inputs['bo'], np.float32)
    outf = np.zeros((B, N, D), np.float32)
    for c in range(NCORES):
        outf[c // 2] += res.results[c]['out']
    outf += bo[None, None, :]
    return outf.reshape(B, T, V, D)



# revision 6
# speedup vs baseline: 1.1293x; 1.1293x over previous
"""Trainium2 Bass kernel for ClinicalStateFormationOperator.

Full-input contract: kernel(**inputs) takes the complete (unsharded) numpy
inputs and returns the full [B, T, V, D] output. Internally the work is
sharded across 8 NeuronCores as (batch, head-group): core c handles batch
c//2 and heads (c%2)*4 .. (c%2)*4+3. Each core computes its 4 heads'
attention and the partial output projection; the host sums the two partial
projections per batch and adds the output bias.

Math notes (per core, N = T*V = 1536 tokens, head_dim=64, obs_dim=16).
Scores are computed transposed (keys on partitions, queries free) in ONE
K=128 matmul per [128k x 512q] tile by packing four contraction groups:

  rows  0: 64  kT_h            |  qT_h * scale         (content; scale is
               folded into Wq on the host -- 1/8 is exact in fp32)
  rows 64: 80  okT_h           |  oqT_h * obs_scale    (observation; both
               sides computed on the host -- a [N,2]@[2,16] expansion, same
               category as the bias gather tables -- and DMA'd into the packs)
  rows 80:112  [K%32==j]       |  VB_h[Q%32, j]        (variable bias)
  rows112:128  A_hj[s,K]=rtb_h[16j+s-K//32+47] | [(Q//32)%16==s]  (time bias;
               the 16-row basis spans the 16 time bins of q-chunk j, so the
               A rows are re-DMA'd into the k-pack once per (head, q-chunk))

  E^T = exp(scores^T)  (no max-subtraction: |scores| <~ 10, fp32-safe)
  [out^T; denom_rep] = [v | ones_64]^T @ E^T  (ones columns replicate the
        softmax denominator across 64 partitions -> aligned divide)
  OT = out^T * reciprocal(denom_rep)
  y_partial = OT^T_heads @ Wo_rows   (host sums core pairs + bo)

All matmuls run in float32r (full-rate fp32 PE mode, ~1e-4 rel err).

Engine budget per core (cost-model cycles): PE ~184k cy @2.4GHz = 76.8us
(projections 37k, scores 74k, attn*V 74k); Act = 48 exp instructions only
(~70us); DVE = q-pack copies + reciprocal + divide (~24us); Pool = k/v/y
copies + ones memsets (~38us). The emission order starts head 0's score
pipeline right after its projections so the Act engine saturates early,
and a short PE warmup loop during the DMA lead-in buys the 2.4GHz p-state
before real work arrives.

q/k/v biases are zero in this problem's setup_inputs; a with-bias variant
(K=1 bias matmuls into the projection psums) is built only if a nonzero
bias is ever passed. boq/bok fold into the host-computed obs rows and bo
is added on the host.
"""

import numpy as np

import concourse.bass as bass
import concourse.mybir as mybir
import concourse.tile as tile
from concourse.bass_utils import run_bass_kernel_spmd

V = 32
T = 48
D = 512
H = 8
HD = D // H          # 64
OD = 16
B = 4
N = T * V            # 1536
HPC = 4              # heads per core
NCORES = 8
SCALE = 1.0 / np.sqrt(HD)      # 1/8, exact in fp32
OBS_SCALE = 1.0 / np.sqrt(OD)  # 1/4, exact in fp32

F32 = mybir.dt.float32
F32R = mybir.dt.float32r
EXP = mybir.ActivationFunctionType.Exp

KC = N // 128        # 12 key chunks of 128
QC = N // 512        # 3 query chunks of 512
GC = KC // 3         # 4 score/exp groups of 3 key chunks per (head, q-chunk)


def _split_waits(nc, max_waits=1):
    """Walrus in this container allows only one sync-wait slot per
    instruction; spill extra waits onto preceding same-engine NoOps."""
    def fix_bb(bb):
        changed = False
        new = []
        for inst in bb.instructions:
            si = inst.sync_info
            if si is not None and len(si.on_wait) > max_waits:
                waits = list(si.on_wait)
                for w in waits[:-max_waits]:
                    new.append(mybir.InstNoOp(
                        name=nc.get_next_instruction_name(),
                        engine=inst.engine, ins=[], outs=[],
                        sync_info=mybir.SyncInfo(on_wait=[w], on_update=[])))
                    changed = True
                si.on_wait = waits[-max_waits:]
            new.append(inst)
        if changed:
            bb.instructions = new
        for sub in getattr(bb, 'blocks', []) or []:
            fix_bb(sub)
    for f in nc.m.functions:
        for bb in f.blocks:
            fix_bb(bb)


def _build(with_bias=False):
    nc = bass.Bass()

    # ---- per-core DRAM I/O (data differs per core, program is SPMD) ----
    fhT = nc.dram_tensor('fhT', [D, N], F32R, kind='ExternalInput')
    wq = nc.dram_tensor('wq', [D, HPC * HD], F32R, kind='ExternalInput')
    wk = nc.dram_tensor('wk', [D, HPC * HD], F32R, kind='ExternalInput')
    wv = nc.dram_tensor('wv', [D, HPC * HD], F32R, kind='ExternalInput')
    wo = nc.dram_tensor('wo', [2, 128, D], F32R, kind='ExternalInput')
    # score-pack static rows (host-built):
    #   qstat[hh] -> q-pack rows 64:128  (oqT*os | VB gather | time one-hot)
    #   kstat[hh] -> k-pack rows 64:112  (okT    | var one-hot)
    #   apack[hh,j] -> k-pack rows 112:128 (time-bias basis per q-chunk)
    qstat = nc.dram_tensor('qstat', [HPC, 64, N], F32R, kind='ExternalInput')
    kstat = nc.dram_tensor('kstat', [HPC, 48, N], F32R, kind='ExternalInput')
    apack = nc.dram_tensor('apack', [HPC, QC, 16, N], F32R,
                           kind='ExternalInput')
    vones = nc.dram_tensor('vones', [128, 512], F32R, kind='ExternalInput')
    if with_bias:
        bqr = nc.dram_tensor('bqr', [1, HPC * HD], F32R, kind='ExternalInput')
        bkr = nc.dram_tensor('bkr', [1, HPC * HD], F32R, kind='ExternalInput')
        bvr = nc.dram_tensor('bvr', [1, HPC * HD], F32R, kind='ExternalInput')
        onesd = nc.dram_tensor('onesd', [1, 512], F32R, kind='ExternalInput')
    out = nc.dram_tensor('out', [N, D], F32, kind='ExternalOutput')

    with tile.TileContext(nc) as tc:
        with tc.tile_pool(name='cst', bufs=1) as cst, \
             tc.tile_pool(name='big', bufs=1) as big, \
             tc.tile_pool(name='work', bufs=3) as work, \
             tc.tile_pool(name='et', bufs=3) as etp, \
             tc.tile_pool(name='ps3', bufs=2, space='PSUM') as ps3, \
             tc.tile_pool(name='ps', bufs=2, space='PSUM') as ps:

            # ---- PE warmup: keep the tensor engine busy through the DMA
            # lead-in so real matmuls start at the 2.4GHz p-state. The warm
            # tile doubles as the ones source for the denominator columns.
            t_warm = cst.tile([128, 512], F32R)
            nc.sync.dma_start(t_warm[:], vones[:])
            p_warm = ps.tile([128, 512], F32, tag='mm', name='p_warm')
            for _ in range(10):
                nc.tensor.matmul(p_warm[:], t_warm[:, 0:128], t_warm[:],
                                 start=True, stop=True)

            # ---- input DMAs, critical-path first, split across the SP and
            # Activation HWDGE queues so transfers overlap.
            t_wq = cst.tile([128, 4, HPC * HD], F32R)
            wq_r = wq[:].rearrange('(o p) n -> p o n', p=128)
            nc.sync.dma_start(t_wq[:, 0:2, :], wq_r[:, 0:2, :])
            nc.scalar.dma_start(t_wq[:, 2:4, :], wq_r[:, 2:4, :])
            t_fhT = big.tile([128, 4, N], F32R)
            fhT_r = fhT[:].rearrange('(o p) n -> p o n', p=128)
            HN = N // 2
            for kk in range(4):
                nc.sync.dma_start(t_fhT[:, kk, 0:HN], fhT_r[:, kk, 0:HN])
                nc.scalar.dma_start(t_fhT[:, kk, HN:N], fhT_r[:, kk, HN:N])

            # score packs per head [128, N]; static rows DMA'd from tables
            t_qp = [big.tile([128, N], F32R, tag=f'qp{hh}', name=f'qp{hh}')
                    for hh in range(HPC)]
            t_kp = [big.tile([128, N], F32R, tag=f'kp{hh}', name=f'kp{hh}')
                    for hh in range(HPC)]
            t_wk = cst.tile([128, 4, HPC * HD], F32R)
            wk_r = wk[:].rearrange('(o p) n -> p o n', p=128)
            nc.scalar.dma_start(t_wk[:, 0:2, :], wk_r[:, 0:2, :])
            nc.sync.dma_start(t_wk[:, 2:4, :], wk_r[:, 2:4, :])
            # head 0's pack statics + first time-bias rows land first
            nc.sync.dma_start(t_kp[0][64:112, :], kstat[0])
            nc.scalar.dma_start(t_qp[0][64:128, :], qstat[0])
            nc.sync.dma_start(t_kp[0][112:128, :], apack[0, 0])
            t_wv = cst.tile([128, 4, HPC * HD], F32R)
            nc.sync.dma_start(t_wv[:], wv[:].rearrange('(o p) n -> p o n',
                                                       p=128))
            for hh in range(1, HPC):
                nc.scalar.dma_start(t_qp[hh][64:128, :], qstat[hh])
                nc.sync.dma_start(t_kp[hh][64:112, :], kstat[hh])
                nc.sync.dma_start(t_kp[hh][112:128, :], apack[hh, 0])
            t_wo = cst.tile([128, 2, D], F32R)
            nc.scalar.dma_start(t_wo[:], wo[:].rearrange('o p n -> p o n'))
            if with_bias:
                t_bq = cst.tile([1, HPC * HD], F32R)
                nc.sync.dma_start(t_bq[:], bqr[:])
                t_bk = cst.tile([1, HPC * HD], F32R)
                nc.sync.dma_start(t_bk[:], bkr[:])
                t_bv = cst.tile([1, HPC * HD], F32R)
                nc.sync.dma_start(t_bv[:], bvr[:])
                t_ones = cst.tile([1, 512], F32R)
                nc.sync.dma_start(t_ones[:], onesd[:])

            # v in natural layout per head + 64 ones columns for denominators
            t_v = [big.tile([128, KC, 128], F32R, tag=f'v{hh}', name=f'v{hh}')
                   for hh in range(HPC)]
            for hh in range(HPC):
                nc.sync.dma_start(
                    t_v[hh][:, :, 64:128],
                    vones[:, None, 0:64].to_broadcast((128, KC, 64)))
            # attention-out^T pairs (heads 2p, 2p+1 stacked on partitions)
            t_ot = [big.tile([128, N], F32R, tag=f'ot{p}', name=f'ot{p}')
                    for p in range(2)]

            # ---- stage 1: projections ----
            # qT / kT: psum rows = 128 output channels (2 heads), cols = tokens
            def emit_qk(m):
                for (w_t, b_name, pack) in ((t_wq, 'bq', t_qp),
                                            (t_wk, 'bk', t_kp)):
                    for j in range(QC):
                        p_qt = ps.tile([128, 512], F32, tag='mm', name='p_qt')
                        for kk in range(4):
                            nc.tensor.matmul(
                                p_qt[:], w_t[:, kk, m * 128:(m + 1) * 128],
                                t_fhT[:, kk, j * 512:(j + 1) * 512],
                                start=(kk == 0),
                                stop=(not with_bias and kk == 3))
                        if with_bias:
                            bt = t_bq if b_name == 'bq' else t_bk
                            nc.tensor.matmul(
                                p_qt[:], bt[:, m * 128:(m + 1) * 128],
                                t_ones[:], start=False, stop=True)
                        for s in range(2):
                            hh = 2 * m + s
                            nc.vector.tensor_copy(
                                pack[hh][0:64, j * 512:(j + 1) * 512],
                                p_qt[s * 64:(s + 1) * 64, :])

            def emit_v(kc0, kc1):
                # v natural layout: psum [128 tokens, 256 ch] per token chunk
                for kc in range(kc0, kc1):
                    p_v = ps.tile([128, HPC * HD], F32, tag='mm', name='p_v')
                    for kk in range(4):
                        nc.tensor.matmul(p_v[:],
                                         t_fhT[:, kk, kc * 128:(kc + 1) * 128],
                                         t_wv[:, kk, :], start=(kk == 0),
                                         stop=(not with_bias and kk == 3))
                    if with_bias:
                        nc.tensor.matmul(p_v[:], t_ones[:, 0:128], t_bv[:],
                                         start=False, stop=True)
                    for hh in range(HPC):
                        nc.vector.tensor_copy(t_v[hh][:, kc, 0:64],
                                              p_v[:, hh * 64:(hh + 1) * 64])

            def emit_scores(hh, j, g, p_s3):
                for i3 in range(3):
                    kc = 3 * g + i3
                    nc.tensor.matmul(
                        p_s3[:, i3, :],
                        t_kp[hh][:, kc * 128:(kc + 1) * 128],
                        t_qp[hh][:, j * 512:(j + 1) * 512],
                        start=True, stop=True)

            def emit_exp(p_s3):
                t_et = etp.tile([128, 3, 512], F32R, tag='et', name='t_et')
                nc.scalar.activation(t_et[:], p_s3[:], EXP)
                return t_et

            def emit_av(hh, g, t_et, p_ot):
                for i3 in range(3):
                    kc = 3 * g + i3
                    nc.tensor.matmul(p_ot[:], t_v[hh][:, kc, :],
                                     t_et[:, i3, :],
                                     start=(kc == 0), stop=(kc == KC - 1))

            def emit_div(hh, j, p_ot):
                t_rec = work.tile([64, 512], F32, tag='rec', name='t_rec')
                nc.vector.reciprocal(t_rec[:], p_ot[64:128, :])
                nc.vector.tensor_mul(
                    t_ot[hh // 2][(hh % 2) * 64:(hh % 2) * 64 + 64,
                                  j * 512:(j + 1) * 512],
                    p_ot[0:64, :], t_rec[:])

            def emit_yout(j):
                # partial out-projection for this q-chunk's 4 row blocks
                for qq in range(4):
                    qc = 4 * j + qq
                    p_y = ps.tile([128, D], F32, tag='mm', name='p_y')
                    for p in range(2):
                        nc.tensor.matmul(p_y[:],
                                         t_ot[p][:, qc * 128:(qc + 1) * 128],
                                         t_wo[:, p, :], start=(p == 0),
                                         stop=(p == 1))
                    t_y = work.tile([128, D], F32, tag='y', name='t_y')
                    nc.vector.tensor_copy(t_y[:], p_y[:])
                    nc.sync.dma_start(out[qc * 128:(qc + 1) * 128, :], t_y[:])

            # ---- emission: head 0's score pipeline starts right after its
            # projections; remaining projection work fills PE slack while
            # the Act engine chews on exp. The Tile list-scheduler reorders
            # within each engine queue as dependencies allow.
            emit_qk(0)                     # heads 0,1 content rows
            sq = {}
            for g in range(2):             # first two score groups of (0,0)
                p_s3 = ps3.tile([128, 3, 512], F32, tag='s3', name='p_s3')
                emit_scores(0, 0, g, p_s3)
                sq[g] = p_s3
            emit_v(0, 6)
            emit_qk(1)                     # heads 2,3 content rows
            emit_v(6, KC)

            for j in range(QC):
                for hh in range(HPC):
                    p_ot = ps.tile([128, 512], F32, tag='mm', name='p_ot')
                    for g in range(GC):
                        if (hh, j) == (0, 0) and g < 2:
                            p_s3 = sq.pop(g)
                        else:
                            p_s3 = ps3.tile([128, 3, 512], F32, tag='s3',
                                            name='p_s3')
                            emit_scores(hh, j, g, p_s3)
                        t_et = emit_exp(p_s3)
                        emit_av(hh, g, t_et, p_ot)
                    if j + 1 < QC:
                        # prefetch next q-chunk's time-bias basis rows; only
                        # WARs this head's just-finished scores, and the next
                        # read is 3 heads (~15us) away
                        nc.sync.dma_start(t_kp[hh][112:128, :],
                                          apack[hh, j + 1])
                    emit_div(hh, j, p_ot)
                emit_yout(j)

    _split_waits(nc)
    return nc


_NC_CACHE = {}


def _get_nc(with_bias=False):
    if with_bias not in _NC_CACHE:
        _NC_CACHE[with_bias] = _build(with_bias)
    return _NC_CACHE[with_bias]


def _host_prep(h, observation_state, Wq, bq, Wk, bk, Wv, bv, Wo, bo,
               Woq, boq, Wok, bok, variable_bias, relative_time_bias,
               with_bias=False):
    f32 = np.float32
    h = np.asarray(h, f32)
    obs = np.asarray(observation_state, f32).reshape(B, N, 2)
    Kidx = np.arange(N)
    tK = Kidx // V                                     # time bin of each token
    varsel = (Kidx[None, :] % V == np.arange(V)[:, None]).astype(f32)
    timesel = ((Kidx[None, :] // V) % 16 == np.arange(16)[:, None]).astype(f32)

    # full observation projections (tiny: [B,N,2] @ [2,128])
    oqT = (obs @ np.asarray(Woq, f32) + np.asarray(boq, f32)) * OBS_SCALE
    okT = obs @ np.asarray(Wok, f32) + np.asarray(bok, f32)
    oqT = oqT.transpose(0, 2, 1)                       # [B, H*OD, N]
    okT = okT.transpose(0, 2, 1)

    in_maps = []
    for c in range(NCORES):
        b, hg = divmod(c, 2)
        h0 = hg * HPC
        cs, ce = h0 * HD, (h0 + HPC) * HD
        qstat = np.empty((HPC, 64, N), f32)
        kst = np.empty((HPC, 48, N), f32)
        ap = np.empty((HPC, QC, 16, N), f32)
        for hh in range(HPC):
            head = h0 + hh
            vb = np.asarray(variable_bias[head], f32)
            rtb = np.asarray(relative_time_bias[head], f32)
            qstat[hh, 0:16] = oqT[b, head * OD:(head + 1) * OD]
            qstat[hh, 16:48] = vb[Kidx % V, :].T       # VB_h[Q%32, j]
            qstat[hh, 48:64] = timesel
            kst[hh, 0:16] = okT[b, head * OD:(head + 1) * OD]
            kst[hh, 16:48] = varsel
            for j in range(QC):
                # A_hj[s, K] = rtb[16j + s - K//32 + 47]
                idx = 16 * j + np.arange(16)[:, None] - tK[None, :] + (T - 1)
                ap[hh, j] = rtb[idx]
        m = {
            'fhT': np.ascontiguousarray(h[b].reshape(N, D).T),
            'wq': np.ascontiguousarray(np.asarray(Wq, f32)[:, cs:ce]) * SCALE,
            'wk': np.ascontiguousarray(np.asarray(Wk, f32)[:, cs:ce]),
            'wv': np.ascontiguousarray(np.asarray(Wv, f32)[:, cs:ce]),
            'wo': np.ascontiguousarray(
                np.asarray(Wo, f32)[cs:ce, :].reshape(2, 128, D)),
            'qstat': qstat,
            'kstat': kst,
            'apack': ap,
            'vones': np.ones((128, 512), np.float32),
        }
        if with_bias:
            m.update({
                'bqr': np.asarray(bq, f32)[None, cs:ce] * SCALE,
                'bkr': np.ascontiguousarray(np.asarray(bk, f32)[None, cs:ce]),
                'bvr': np.ascontiguousarray(np.asarray(bv, f32)[None, cs:ce]),
                'onesd': np.ones((1, 512), f32),
            })
        in_maps.append(m)
    return in_maps


def kernel(**inputs):
    with_bias = any(
        np.any(np.asarray(inputs[k])) for k in ('bq', 'bk', 'bv'))
    nc = _get_nc(with_bias)
    in_maps = _host_prep(**inputs, with_bias=with_bias)
    res = run_bass_kernel_spmd(nc, in_maps, core_ids=list(range(NCORES)))
    bo = np.asarray(inputs['bo'], np.float32)
    outf = np.zeros((B, N, D), np.float32)
    for c in range(NCORES):
        outf[c // 2] += res.results[c]['out']
    outf += bo[None, None, :]
    return outf.reshape(B, T, V, D)
